# revision 11
# baseline (speedup 1.0000x reference)
"""GATv2 localization model on 8 Trainium2 NeuronCores (Bass/Tile) — v2.

Strategy (dst-sharded message passing, bf16 pipeline):
  - Nodes sharded across 8 cores by dst (6250 each); edges live with their
    dst core. Per core, nodes are degree-sorted into 49 blocks of 128.
  - Per block, incoming edges are packed into slot columns; blocks are
    processed in passes of pow2 width w<=16 (variable-width tail passes).
  - Layer-1 channels are (c-outer, h-inner) interleaved so per-head
    broadcasts run in the DVE 2x bf16 mode; layer 2 uses 4 fake heads.
  - Per pass: PE builds prefill = xr[dst] + We*ea (identity + block-diag
    matmuls) and accumulates the gathered xl[src] rows into PSUM; ACT
    applies leaky-relu straight out of PSUM; DVE does the att-dot
    (tree-reduce), unnormalized softmax weights, weighted slot sums.
  - Softmax is computed unnormalized (exp(logit), no max subtraction);
    masked pad slots contribute exactly zero.
  - Layer-1 tables (xl1 = x@Wl1.T + bl1, per-block xr1) are host-computed
    in bf16; layer-2 tables are built on-device from h1 and exchanged
    through the host between the two launches. MLP head on-device.
"""

import os
import numpy as np
import ml_dtypes

import concourse.bacc as bacc
import concourse.tile as tile
import concourse.mybir as mybir
from concourse import bass
from concourse.bass_utils import run_bass_kernel_spmd
from concourse.masks import make_identity

F32 = mybir.dt.float32
BF16 = mybir.dt.bfloat16
I32 = mybir.dt.int32
BF = ml_dtypes.bfloat16

N = 50000
E = 800000
IN = 16
H1 = 4
C1 = 32
HC = 128
OUT = 2
NCORES = 8
NSHARD = N // NCORES          # 6250
NBLK = (NSHARD + 127) // 128  # 49
NPAD = NBLK * 128             # 6272
K = 16                        # max slots per pass

PREGATHER = os.environ.get("GAT_PREGATHER", "1") == "1"

_EXEC_NS = []                 # per-launch HW exec time when GAT_TRACE=1


def _maybe_install_trace_hook():
    if os.environ.get("GAT_TRACE", "0") != "1":
        return False
    import contextlib, ctypes, sys, types
    if "antenv.axon_hooks" not in sys.modules:
        def _mk(so_path):
            lib = ctypes.CDLL(so_path)
            if not hasattr(lib, "axon_start_nrt_profile"):
                return None
            lib.axon_start_nrt_profile.argtypes = [ctypes.POINTER(ctypes.c_int64), ctypes.c_size_t]
            lib.axon_start_nrt_profile.restype = ctypes.c_int64
            lib.axon_stop_nrt_profile.argtypes = [ctypes.c_char_p]
            lib.axon_stop_nrt_profile.restype = ctypes.c_int64

            @contextlib.contextmanager
            def _hook(output_dir, device_ids):
                import jax
                jax.devices()
                if device_ids:
                    ids = (ctypes.c_int64 * len(device_ids))(*device_ids)
                    rc = lib.axon_start_nrt_profile(ids, len(device_ids))
                else:
                    rc = lib.axon_start_nrt_profile(None, 0)
                if rc != 0:
                    raise RuntimeError(f"axon_start_nrt_profile rc={rc}")
                try:
                    yield
                finally:
                    n = lib.axon_stop_nrt_profile(str(output_dir).encode())
                    if n < 0:
                        raise RuntimeError(f"axon_stop_nrt_profile rc={n}")
            return _hook

        hook = _mk("/opt/axon/libaxon_pjrt.so")
        mod = types.ModuleType("antenv.axon_hooks")
        mod.get_axon_ntff_profile_hook = lambda: hook
        mod.set_axon_ntff_profile_hook = lambda h: None
        sys.modules["antenv.axon_hooks"] = mod
        import concourse.bass_utils as bu
        bu.upload_artifacts = lambda tmpdir: tmpdir
    return True


def _run(nc, in_maps):
    trace = _maybe_install_trace_hook()
    if trace:
        import tempfile
        res = run_bass_kernel_spmd(nc, in_maps, core_ids=list(range(NCORES)),
                                   trace=True, tmpdir=tempfile.mkdtemp())
        _EXEC_NS.append(res.exec_time_ns)
    else:
        res = run_bass_kernel_spmd(nc, in_maps, core_ids=list(range(NCORES)))
    return res.results


# ---------------------------------------------------------------- schedule

def _pow2_parts(s):
    """Decompose s into <=3 descending pow2 widths <=16 (round up tail)."""
    parts = []
    while s > 0 and len(parts) < 2:
        w = 16
        while w > s:
            w //= 2
        parts.append(w)
        s -= w
    if s > 0:
        w = 1
        while w < s:
            w *= 2
        parts.append(min(w, 16))
    return parts


def _build_schedule(edge_index, edge_attr):
    src = edge_index[0].astype(np.int64)
    dst = edge_index[1].astype(np.int64)
    ea = edge_attr[:, 0].astype(np.float32)
    deg = np.bincount(dst, minlength=N)
    e_order = np.argsort(dst, kind="stable")
    src_s = src[e_order].astype(np.int32)
    ea_s = ea[e_order]
    starts = np.searchsorted(dst[e_order], np.arange(N + 1))

    SLOTS = np.zeros(NBLK, np.int64)
    perms = []
    for k in range(NCORES):
        lo = k * NSHARD
        order = np.argsort(-deg[lo:lo + NSHARD], kind="stable")
        perm = lo + order                       # block row r -> global node id
        perms.append(perm)
        dpad = np.concatenate([deg[perm], np.zeros(NPAD - NSHARD, np.int64)])
        SLOTS = np.maximum(SLOTS, dpad.reshape(NBLK, 128).max(1))
    SLOTS = np.maximum(SLOTS, 1)

    passes = []      # (block, s0, w)
    blk_passes = []  # per block: list of pass indices
    for b in range(NBLK):
        parts = _pow2_parts(int(SLOTS[b]))
        s0 = 0
        pl = []
        for w in parts:
            pl.append(len(passes))
            passes.append((b, s0, w))
            s0 += w
        blk_passes.append(pl)
    NPASS = len(passes)

    SMAX = int(SLOTS.max())
    pass_of = np.zeros((NBLK, SMAX), np.int64)
    j_of = np.zeros((NBLK, SMAX), np.int64)
    for b in range(NBLK):
        for pi in blk_passes[b]:
            _, s0, w = passes[pi]
            hi = min(s0 + w, int(SLOTS[b]))
            if hi > s0:
                pass_of[b, s0:hi] = pi
                j_of[b, s0:hi] = np.arange(hi - s0)

    cores = []
    for k in range(NCORES):
        lo = k * NSHARD
        perm = perms[k]
        order = perm - lo
        rowpos = np.empty(NSHARD, np.int64)
        rowpos[order] = np.arange(NSHARD)
        es, ee = starts[lo], starts[lo + NSHARD]
        nid = dst[e_order][es:ee] - lo
        rank = np.arange(es, ee) - starts[lo + nid]
        r = rowpos[nid]
        b = r >> 7
        p = r & 127
        pi = pass_of[b, rank]
        j = j_of[b, rank]

        idx_all = np.zeros((NPASS, 128, K), np.int32)
        eam = np.zeros((NPASS, 128, 2 * K), BF)
        eaT = np.zeros((NPASS, 4, 4 * 128), BF)
        idx_all[pi, p, j] = src_s[es:ee]
        eam[pi, p, j] = ea_s[es:ee]
        eam[pi, p, K + j] = 1.0
        eaT[pi, j & 3, (j >> 2) * 128 + p] = ea_s[es:ee]
        meta = np.zeros((NPASS, 128, 32), np.int32)
        meta[:, :, 0:16] = idx_all
        meta[:, :, 16:32] = eam.view(np.uint16).reshape(NPASS, 128, 2 * K) \
            .view(np.int32)
        cores.append({"perm": perm, "meta": meta, "idx": idx_all, "eaT": eaT})
    return cores, passes, blk_passes, NPASS


def _slot_offsets(passes):
    """Flat row offsets of each pass in the pregathered slot stream."""
    offs, tot = [], 0
    for (_, _, w) in passes:
        offs.append(tot)
        tot += 128 * w
    return offs, tot


def _pregather_stream(xlb, idx_all, passes, offs, tot):
    """Host gather: xlb[idx] packed p-major per pass -> [tot, HC] bf16."""
    out = np.empty((tot, HC), BF)
    for pi, (_, _, w) in enumerate(passes):
        o = offs[pi]
        out[o:o + 128 * w] = xlb[idx_all[pi, :, :w]].reshape(128 * w, HC)
    return out


# ---------------------------------------------------------------- launches

def _build_launch(layer, passes, blk_passes, NPASS, offs, tot):
    nc = bacc.Bacc("TRN2", target_bir_lowering=False, debug=False,
                   num_devices=NCORES)
    L1 = layer == 1

    t_xl = nc.dram_tensor("t_xl", [N, HC], BF16, kind="ExternalInput")
    t_xrb = nc.dram_tensor("t_xrb", [NBLK, 128, HC], BF16, kind="ExternalInput")
    t_meta = nc.dram_tensor("t_meta", [NPASS, 128, 32], I32, kind="ExternalInput")
    t_eaT = nc.dram_tensor("t_eaT", [NPASS, 4, 512], BF16, kind="ExternalInput")
    t_we4 = nc.dram_tensor("t_we4", [4, 4 * HC], BF16, kind="ExternalInput")
    t_attb = nc.dram_tensor("t_attb", [128, HC], BF16, kind="ExternalInput")
    t_brow = nc.dram_tensor("t_brow", [128, HC], F32, kind="ExternalInput")
    if PREGATHER:
        t_slot = nc.dram_tensor("t_slot", [tot, HC], BF16, kind="ExternalInput")
    if L1:
        t_wl2 = nc.dram_tensor("t_wl2", [HC, HC], F32, kind="ExternalInput")
        t_wr2 = nc.dram_tensor("t_wr2", [HC, HC], F32, kind="ExternalInput")
        t_bl2row = nc.dram_tensor("t_bl2row", [128, HC], F32, kind="ExternalInput")
        t_br2row = nc.dram_tensor("t_br2row", [128, HC], F32, kind="ExternalInput")
        o_xl2 = nc.dram_tensor("o_xl2", [NPAD, HC], BF16, kind="ExternalOutput")
        o_xr2 = nc.dram_tensor("o_xr2", [NBLK, 128, HC], BF16, kind="ExternalOutput")
    else:
        t_w1 = nc.dram_tensor("t_w1", [HC, 32], F32, kind="ExternalInput")
        t_w2 = nc.dram_tensor("t_w2", [32, 32], F32, kind="ExternalInput")
        t_w3 = nc.dram_tensor("t_w3", [32, OUT], F32, kind="ExternalInput")
        t_c1 = nc.dram_tensor("t_c1", [32, 1], F32, kind="ExternalInput")
        t_c2 = nc.dram_tensor("t_c2", [32, 1], F32, kind="ExternalInput")
        t_c3 = nc.dram_tensor("t_c3", [OUT, 1], F32, kind="ExternalInput")
        o_out = nc.dram_tensor("o_out", [NBLK, OUT, 128], F32, kind="ExternalOutput")

    PRELU = mybir.ActivationFunctionType.Prelu
    EXPF = mybir.ActivationFunctionType.Exp
    RELU = mybir.ActivationFunctionType.Relu

    with tile.TileContext(nc) as tc:
        with tc.tile_pool(name="const", bufs=1) as cpool, \
             tc.tile_pool(name="blk", bufs=3) as bpool, \
             tc.tile_pool(name="pas", bufs=4) as ppool, \
             tc.tile_pool(name="psum", bufs=2, space="PSUM") as psum:
            we4 = cpool.tile([4, 4 * HC], BF16)
            nc.sync.dma_start(out=we4[:], in_=t_we4.ap())
            attb = cpool.tile([128, HC], BF16)
            nc.sync.dma_start(out=attb[:], in_=t_attb.ap())
            brow = cpool.tile([128, HC], F32)
            nc.sync.dma_start(out=brow[:], in_=t_brow.ap())
            ident = cpool.tile([128, 128], F32)
            make_identity(nc, ident[:])
            identb = cpool.tile([128, 128], BF16)
            nc.vector.tensor_copy(out=identb[:], in_=ident[:])
            if L1:
                wl2 = cpool.tile([HC, HC], F32)
                nc.sync.dma_start(out=wl2[:], in_=t_wl2.ap())
                wr2 = cpool.tile([HC, HC], F32)
                nc.sync.dma_start(out=wr2[:], in_=t_wr2.ap())
                bl2row = cpool.tile([128, HC], F32)
                nc.sync.dma_start(out=bl2row[:], in_=t_bl2row.ap())
                br2row = cpool.tile([128, HC], F32)
                nc.sync.dma_start(out=br2row[:], in_=t_br2row.ap())
            else:
                w1 = cpool.tile([HC, 32], F32)
                nc.sync.dma_start(out=w1[:], in_=t_w1.ap())
                w2 = cpool.tile([32, 32], F32)
                nc.sync.dma_start(out=w2[:], in_=t_w2.ap())
                w3 = cpool.tile([32, OUT], F32)
                nc.sync.dma_start(out=w3[:], in_=t_w3.ap())
                c1 = cpool.tile([32, 1], F32)
                nc.sync.dma_start(out=c1[:], in_=t_c1.ap())
                c2 = cpool.tile([32, 1], F32)
                nc.sync.dma_start(out=c2[:], in_=t_c2.ap())
                c3 = cpool.tile([OUT, 1], F32)
                nc.sync.dma_start(out=c3[:], in_=t_c3.ap())

            for b in range(NBLK):
                xrb = bpool.tile([128, HC], BF16, tag="xrb")
                nc.sync.dma_start(out=xrb[:], in_=t_xrb.ap()[b])
                accum = bpool.tile([128, 4, HC], F32, tag="accum", bufs=3)
                nc.vector.memset(accum[:], 0.0)
                dacc = bpool.tile([128, 4], F32, tag="dacc", bufs=3)
                nc.vector.memset(dacc[:], 1e-20)

                for pl, pi in enumerate(blk_passes[b]):
                    _, s0, w = passes[pi]
                    meta = ppool.tile([128, 32], I32, tag="meta")
                    nc.sync.dma_start(out=meta[:], in_=t_meta.ap()[pi])
                    eaT = ppool.tile([4, 512], BF16, tag="eaT")
                    nc.sync.dma_start(out=eaT[:], in_=t_eaT.ap()[pi])
                    mk = meta[:, 16:32].bitcast(BF16)[:, K:K + w]

                    xlg = ppool.tile([128, K, HC], BF16, tag="xlg")
                    if PREGATHER:
                        o = offs[pi]
                        eng = nc.sync if pi % 2 == 0 else nc.scalar
                        eng.dma_start(
                            out=xlg[:, :w, :],
                            in_=t_slot.ap()[o:o + 128 * w]
                                .rearrange("(p j) c -> p j c", p=128))
                    else:
                        for j in range(w):
                            nc.gpsimd.indirect_dma_start(
                                out=xlg[:, j, :], out_offset=None, in_=t_xl.ap(),
                                in_offset=bass.IndirectOffsetOnAxis(
                                    ap=meta[:, j:j + 1], axis=0))

                    # prefill + gathered rows through PE into PSUM, prelu out
                    m = ppool.tile([128, K, HC], BF16, tag="m")
                    nch = (w + 3) // 4
                    for q in range(nch):
                        cw = min(4, w - 4 * q)
                        pf = psum.tile([128, 4, HC], F32, tag="pf", bufs=5 if L1 else 4)
                        nc.tensor.matmul(
                            out=pf[:, :cw, :],
                            lhsT=eaT[0:cw, 128 * q:128 * (q + 1)],
                            rhs=we4[0:cw, 0:cw * HC],
                            start=True, stop=False)
                        nc.tensor.matmul(
                            out=pf[:, :cw, :], lhsT=identb[:],
                            rhs=xrb[:].unsqueeze(1).broadcast_to([128, cw, HC]),
                            start=False, stop=False)
                        nc.tensor.matmul(
                            out=pf[:, :cw, :], lhsT=identb[:],
                            rhs=xlg[:, 4 * q:4 * q + cw, :],
                            start=False, stop=True)
                        nc.scalar.activation(
                            out=m[:, 4 * q:4 * q + cw, :], in_=pf[:, :cw, :],
                            func=PRELU, alpha=0.2)

                    # logits: m *= att, tree-reduce over c
                    nc.vector.tensor_mul(
                        out=m[:, :w, :], in0=m[:, :w, :],
                        in1=attb[:].unsqueeze(1).broadcast_to([128, w, HC]))
                    if L1:
                        mv = m[:].rearrange("p j (c h) -> p j c h", h=4)
                        c = 32
                        while c > 1:
                            c //= 2
                            nc.vector.tensor_add(
                                out=mv[:, :w, 0:c, :], in0=mv[:, :w, 0:c, :],
                                in1=mv[:, :w, c:2 * c, :])
                        lgap = mv[:, :w, 0:1, :]               # [128, w, 1, 4]
                    else:
                        c = 128
                        while c > 1:
                            c //= 2
                            nc.vector.tensor_add(
                                out=m[:, :w, 0:c], in0=m[:, :w, 0:c],
                                in1=m[:, :w, c:2 * c])
                        lgap = m[:, :w, 0:1]

                    wt = ppool.tile([128, K, 4], BF16, tag="wt")
                    if L1:
                        nc.scalar.activation(out=wt[:, :w, :], in_=lgap, func=EXPF)
                    else:
                        wtn = ppool.tile([128, K], BF16, tag="wtn")
                        nc.scalar.activation(out=wtn[:, :w], in_=lgap, func=EXPF)
                        nc.vector.tensor_copy(
                            out=wt[:, :w, :],
                            in_=wtn[:, :w].unsqueeze(2).broadcast_to([128, w, 4]))
                    # mask pad slots
                    nc.vector.tensor_mul(
                        out=wt[:, :w, :], in0=wt[:, :w, :],
                        in1=mk.unsqueeze(2).broadcast_to([128, w, 4]))
                    # denominators
                    dnp = ppool.tile([128, 4], F32, tag="dnp")
                    nc.vector.tensor_reduce(
                        out=dnp[:], in_=wt[:, :w, :].rearrange("p j h -> p h j"),
                        axis=mybir.AxisListType.X, op=mybir.AluOpType.add)
                    eng_d = nc.gpsimd if PREGATHER else nc.vector
                    eng_d.tensor_add(out=dacc[:], in0=dacc[:], in1=dnp[:])
                    # weighted slot rows: xlg *= wt (h-inner broadcast)
                    nc.vector.tensor_mul(
                        out=xlg[:, :w, :].rearrange("p j (c h) -> p j c h", h=4),
                        in0=xlg[:, :w, :].rearrange("p j (c h) -> p j c h", h=4),
                        in1=wt[:, :w, :].unsqueeze(2).broadcast_to([128, w, 32, 4]))
                    # tree-reduce over slots to width 4, accumulate in f32
                    c = w
                    while c > 4:
                        c //= 2
                        nc.vector.tensor_add(
                            out=xlg[:, 0:c, :], in0=xlg[:, 0:c, :],
                            in1=xlg[:, c:2 * c, :])
                    nc.vector.tensor_add(out=accum[:, 0:c, :],
                                         in0=accum[:, 0:c, :],
                                         in1=xlg[:, 0:c, :])

                # ---- finalize block
                nc.vector.tensor_add(out=accum[:, 0:2, :], in0=accum[:, 0:2, :],
                                     in1=accum[:, 2:4, :])
                nc.vector.tensor_add(out=accum[:, 0, :], in0=accum[:, 0, :],
                                     in1=accum[:, 1, :])
                rec = bpool.tile([128, 4], F32, tag="rec")
                nc.vector.reciprocal(out=rec[:], in_=dacc[:])
                hblk = bpool.tile([128, HC], F32, tag="hblk")
                nc.vector.tensor_mul(
                    out=hblk[:].rearrange("p (c h) -> p c h", h=4),
                    in0=accum[:, 0, :].rearrange("p (c h) -> p c h", h=4),
                    in1=rec[:].unsqueeze(1).broadcast_to([128, 32, 4]))
                nc.vector.tensor_add(out=hblk[:], in0=hblk[:], in1=brow[:])
                # ELU' = relu(x) + exp(min(x,0))   (the -1 is folded downstream)
                tneg = bpool.tile([128, HC], F32, tag="tneg")
                nc.vector.tensor_scalar_min(out=tneg[:], in0=hblk[:], scalar1=0.0)
                nc.scalar.activation(out=tneg[:], in_=tneg[:], func=EXPF)
                nc.scalar.activation(out=hblk[:], in_=hblk[:], func=RELU)
                nc.vector.tensor_add(out=hblk[:], in0=hblk[:], in1=tneg[:])

                # ---- per-block tail
                tp = psum.tile([128, 128], F32, tag="po", bufs=3 if L1 else 1)
                nc.tensor.transpose(out=tp[:], in_=hblk[:], identity=ident[:])
                hT = bpool.tile([128, 128], F32, tag="hT")
                nc.scalar.copy(out=hT[:], in_=tp[:])
                if L1:
                    mm2 = psum.tile([128, 128], F32, tag="po", bufs=3)
                    nc.tensor.matmul(out=mm2[:], lhsT=hT[:], rhs=wl2[:],
                                     start=True, stop=True)
                    xl2sb = bpool.tile([128, HC], BF16, tag="xl2sb")
                    nc.vector.tensor_add(out=xl2sb[:], in0=mm2[:], in1=bl2row[:])
                    nc.sync.dma_start(out=o_xl2.ap()[b * 128:(b + 1) * 128, :],
                                      in_=xl2sb[:])
                    mm3 = psum.tile([128, 128], F32, tag="po", bufs=3)
                    nc.tensor.matmul(out=mm3[:], lhsT=hT[:], rhs=wr2[:],
                                     start=True, stop=True)
                    xr2sb = bpool.tile([128, HC], BF16, tag="xr2sb")
                    nc.vector.tensor_add(out=xr2sb[:], in0=mm3[:], in1=br2row[:])
                    nc.sync.dma_start(out=o_xr2.ap()[b], in_=xr2sb[:])
                else:
                    mp1 = psum.tile([32, 128], F32, tag="mpo", bufs=2)
                    nc.tensor.matmul(out=mp1[:], lhsT=w1[:], rhs=hT[:],
                                     start=True, stop=True)
                    r1 = bpool.tile([32, 128], F32, tag="r1")
                    nc.scalar.activation(out=r1[:], in_=mp1[:], func=RELU,
                                         bias=c1[:, 0:1])
                    mp2 = psum.tile([32, 128], F32, tag="mpo", bufs=2)
                    nc.tensor.matmul(out=mp2[:], lhsT=w2[:], rhs=r1[:],
                                     start=True, stop=True)
                    r2 = bpool.tile([32, 128], F32, tag="r2")
                    nc.scalar.activation(out=r2[:], in_=mp2[:], func=RELU,
                                         bias=c2[:, 0:1])
                    mp3 = psum.tile([OUT, 128], F32, tag="mp3", bufs=1)
                    nc.tensor.matmul(out=mp3[:], lhsT=w3[:], rhs=r2[:],
                                     start=True, stop=True)
                    r3 = bpool.tile([OUT, 128], F32, tag="r3")
                    nc.vector.tensor_scalar_add(out=r3[:], in0=mp3[:],
                                                scalar1=c3[:, 0:1])
                    nc.sync.dma_start(out=o_out.ap()[b], in_=r3[:])
    nc.compile()
    return nc


# ---------------------------------------------------------------- kernel

def _perm_l1():
    """Column permutation: new col (c,h) h-inner <- old col h*32+c."""
    newcols = np.arange(HC)
    c, h = newcols // 4, newcols % 4
    return h * 32 + c


def kernel(x, edge_index, edge_attr,
           Wl1, bl1, Wr1, br1, We1, att1, b1,
           Wl2, bl2, Wr2, br2, We2, att2, b2,
           W1, c1, W2, c2, W3, c3):
    x = np.asarray(x, np.float32)
    edge_index = np.asarray(edge_index, np.int32)
    edge_attr = np.asarray(edge_attr, np.float32)
    f = lambda a: np.asarray(a, np.float32)
    Wl1, bl1, Wr1, br1, We1 = f(Wl1), f(bl1), f(Wr1), f(br1), f(We1)
    att1, b1 = f(att1), f(b1)
    Wl2, bl2, Wr2, br2, We2 = f(Wl2), f(bl2), f(Wr2), f(br2), f(We2)
    att2, b2 = f(att2), f(b2)
    W1, c1, W2, c2, W3, c3 = f(W1), f(c1), f(W2), f(c2), f(W3), f(c3)

    cores, passes, blk_passes, NPASS = _build_schedule(edge_index, edge_attr)
    offs, tot = _slot_offsets(passes)

    pi1 = _perm_l1()                       # L1 column interleave
    row = lambda v: np.tile(np.asarray(v, np.float32)[None, :], (128, 1))

    def we4_of(wef):
        m = np.zeros((4, 4 * HC), np.float32)
        for j in range(4):
            m[j, j * HC:(j + 1) * HC] = wef
        return m.astype(BF)

    # host-side layer-1 tables (interleaved columns)
    xl1 = (x @ Wl1.T + bl1)[:, pi1]
    xr1 = (x @ Wr1.T + br1)[:, pi1]
    xl1b = xl1.astype(BF)
    att1f = att1.reshape(-1)[pi1]
    we1f = We1[:, 0][pi1]
    att2f = att2.reshape(-1)
    we2f = We2[:, 0]
    b1row = row(b1[pi1])
    b2row = row(b2)
    bl2row = row(bl2 - Wl2.sum(axis=1))    # folds ELU's -1
    br2row = row(br2 - Wr2.sum(axis=1))
    c1p = (c1 - W1.sum(axis=1)).reshape(32, 1)
    wl2T = Wl2.T[pi1, :].copy()            # [HC_in(perm), HC_out]
    wr2T = Wr2.T[pi1, :].copy()

    ncA = _build_launch(1, passes, blk_passes, NPASS, offs, tot)
    in_maps = []
    for k in range(NCORES):
        perm = cores[k]["perm"]
        xrb = np.zeros((NPAD, HC), np.float32)
        xrb[:NSHARD] = xr1[perm]
        im = {
            "t_xl": xl1b, "t_xrb": xrb.astype(BF).reshape(NBLK, 128, HC),
            "t_meta": cores[k]["meta"], "t_eaT": cores[k]["eaT"],
            "t_we4": we4_of(we1f), "t_attb": row(att1f).astype(BF),
            "t_brow": b1row,
            "t_wl2": wl2T, "t_wr2": wr2T,
            "t_bl2row": bl2row, "t_br2row": br2row,
        }
        if PREGATHER:
            im["t_slot"] = _pregather_stream(xl1b, cores[k]["idx"], passes,
                                             offs, tot)
        in_maps.append(im)
    resA = _run(ncA, in_maps)

    # exchange through host: assemble natural-order layer-2 tables
    xl2 = np.zeros((N, HC), BF)
    xr2 = np.zeros((N, HC), np.float32)
    for k in range(NCORES):
        perm = cores[k]["perm"]
        xl2[perm] = np.asarray(resA[k]["o_xl2"])[:NSHARD]
        xr2[perm] = np.asarray(resA[k]["o_xr2"]).reshape(NPAD, HC)[:NSHARD] \
            .astype(np.float32)

    ncB = _build_launch(2, passes, blk_passes, NPASS, offs, tot)
    in_mapsB = []
    for k in range(NCORES):
        perm = cores[k]["perm"]
        xrb = np.zeros((NPAD, HC), np.float32)
        xrb[:NSHARD] = xr2[perm]
        im = {
            "t_xl": xl2, "t_xrb": xrb.astype(BF).reshape(NBLK, 128, HC),
            "t_meta": cores[k]["meta"], "t_eaT": cores[k]["eaT"],
            "t_we4": we4_of(we2f), "t_attb": row(att2f).astype(BF),
            "t_brow": b2row,
            "t_w1": W1.T.copy(), "t_w2": W2.T.copy(),
            "t_w3": W3.T.copy(),
            "t_c1": c1p, "t_c2": c2.reshape(32, 1), "t_c3": c3.reshape(OUT, 1),
        }
        if PREGATHER:
            im["t_slot"] = _pregather_stream(xl2, cores[k]["idx"], passes,
                                             offs, tot)
        in_mapsB.append(im)
    resB = _run(ncB, in_mapsB)

    out = np.zeros((N, OUT), np.float32)
    for k in range(NCORES):
        perm = cores[k]["perm"]
        o = np.asarray(resB[k]["o_out"]).transpose(0, 2, 1).reshape(NPAD, OUT)
        out[perm] = o[:NSHARD]
    return out
